# revision 16
# baseline (speedup 1.0000x reference)
"""Trainium2 Bass kernel: scatter rho[b, i, j] -> out[b, fock_idx[i], fock_idx[j]].

Sharding: batch dim B across the 8 NeuronCores (pure data parallel). fock_idx is
known on the host at call time, so the scatter addressing is baked into the
compiled program as static DMA/compute access patterns.

Per-core algorithm (out is [D, D], zero except out[idx[i], idx[j]] = rho[i, j]):
  - The runtime hands the NEFF a zero-initialized ExternalOutput buffer, so
    only rows/columns that receive data are written.
  - fock_idx decomposes into 32 runs of 32 consecutive indices spanning
    [c0, c1).  Each rho row is expanded into a [span]-wide row in SBUF with
    the runs at their target offsets and zeros in the gaps; each out row-run
    is stored with one DMA touching columns [c0, c1) only.
  - The 16 DMA engines (~22.5 GB/s each, shared by all queues) are the
    bottleneck: ~4.2 MB of loads + ~8.3 MB of span stores = ~34.6 us of
    engine time.  To keep them saturated end-to-end, ALL loads are issued
    up front: tiles 0-1 on the Pool SWDGE queue (only two, because SWDGE
    descriptor generation runs on the Q7 cores and would serialize with
    GpSimd's copies), the rest interleaved onto the two HWDGE rings ahead
    of the stores.  A single SWDGE queue feeds 4 KB descriptors at only
    ~160 GB/s, which is why the baseline's loads starved the pipeline.
  - The W expansion buffers are memset once up front and reused cyclically:
    the gap columns stay zero across reuse because the per-tile copies only
    ever write the (fixed) data columns.  Expansion copies run on Vector
    (single runs) and GpSimd (pair-merged runs); stores alternate between
    the two HWDGE rings.
"""

import numpy as np

import concourse.bacc as bacc
import concourse.bass as bass
import concourse.mybir as mybir
from concourse import tile
from concourse.bass_utils import run_bass_kernel_spmd

N_CORES = 8
P = 128  # SBUF partitions
W_BUFS = 6


def _runs(dst, src):
    """Maximal runs where dst and src both advance by 1. Yields (d0, s0, len)."""
    out = []
    d0, s0, L = int(dst[0]), int(src[0]), 1
    for k in range(1, len(dst)):
        if int(dst[k]) == d0 + L and int(src[k]) == s0 + L:
            L += 1
        else:
            out.append((d0, s0, L))
            d0, s0, L = int(dst[k]), int(src[k]), 1
    out.append((d0, s0, L))
    return out


def _pair_runs(col_runs):
    """Group adjacent equal-length runs into stride-2 pairs.

    Returns a list of (dst0, src0, pair_dst_stride, pair_src_stride, n, L)
    where n is 1 or 2 repeats of an L-wide copy.
    """
    out = []
    k = 0
    while k < len(col_runs):
        d0, s0, L = col_runs[k]
        if k + 1 < len(col_runs) and col_runs[k + 1][2] == L:
            d1, s1, _ = col_runs[k + 1]
            out.append((d0, s0, d1 - d0, s1 - s0, 2, L))
            k += 2
        else:
            out.append((d0, s0, L, L, 1, L))
            k += 1
    return out


def _build(idx, D, n):
    """Build the per-core Bass program with idx baked in."""
    f32 = mybir.dt.float32

    # Column placement: process columns in sorted-index order so the SBUF row
    # image is written left to right; a run needs source columns contiguous too.
    order = np.argsort(idx, kind="stable")
    col_runs = _runs(idx[order], order)  # (dst_col, src_col, len)
    c0 = min(r[0] for r in col_runs)
    c1 = max(r[0] + r[2] for r in col_runs)
    span = c1 - c0

    # All copies pair-merged (2 runs per instruction); 13/16 pairs to
    # Vector (~105 ns each measured) and 3/16 to GpSimd (~390 ns each) so
    # the per-tile staging pace (~1.4 us) feeds stores well above the DMA
    # engines' drain rate.
    all_pairs = _pair_runs(col_runs)
    ncut = (len(all_pairs) * 13 + 15) // 16
    pairs_v = all_pairs[:ncut]
    pairs_g = all_pairs[ncut:]

    nc = bacc.Bacc("TRN2", target_bir_lowering=False, debug=False,
                   num_devices=N_CORES)
    rho = nc.dram_tensor("rho", [n, n], f32, kind="ExternalInput")
    out = nc.dram_tensor("out", [D, D], f32, kind="ExternalOutput")

    n_tiles = (n + P - 1) // P
    with tile.TileContext(nc) as tc:
        with (
            tc.tile_pool(name="rp", bufs=1) as rp,
            tc.tile_pool(name="wp", bufs=1) as wp,
        ):
            ws = [wp.tile([P, span], f32, name=f"W{k}") for k in range(W_BUFS)]
            memset_eng = [nc.vector if k % 2 == 0 else nc.gpsimd
                          for k in range(W_BUFS)]

            # One R buffer per tile — every load is in flight at once.
            Rts = [rp.tile([P, n], f32, name=f"R{t}") for t in range(n_tiles)]

            def issue_load(t, eng):
                r0 = t * P
                rows = min(P, n - r0)
                eng.dma_start(Rts[t][:rows, :], rho[r0:r0 + rows, :])

            # All loads up front, first in each HWDGE ring's FIFO (before
            # any stores).  No SWDGE: its descriptor generation runs on the
            # Q7 cores and both serializes with GpSimd's copies and feeds
            # packets at only ~115 GB/s.
            for t in range(n_tiles):
                issue_load(t, nc.sync if t % 2 == 0 else nc.scalar)

            # Memsets after load issue so they do not delay the queues.
            memset_eng[0].memset(ws[0][:], 0.0)
            memset_eng[1].memset(ws[1][:], 0.0)
            next_memset = 2

            n_store = 0
            for t in range(n_tiles):
                r0 = t * P
                rows = min(P, n - r0)
                R = Rts[t]

                W = ws[t % W_BUFS]
                for eng, plist in ((nc.vector, pairs_v), (nc.gpsimd, pairs_g)):
                    for d0, s0, ds, ss, cnt, L in plist:
                        dst = bass.AP(W.tensor, W.offset + (d0 - c0),
                                      [[W.ap[0][0], rows], [ds, cnt], [1, L]])
                        src = bass.AP(R.tensor, R.offset + s0,
                                      [[R.ap[0][0], rows], [ss, cnt], [1, L]])
                        eng.tensor_copy(dst, src)

                # Row runs within this tile: consecutive rho rows with
                # consecutive target rows share one store DMA, alternating
                # between the SP and ACT HWDGE rings.
                for dr, sr, L in _runs(idx[r0:r0 + rows], range(rows)):
                    ring = nc.sync if n_store % 2 == 0 else nc.scalar
                    n_store += 1
                    ring.dma_start(out[dr:dr + L, c0:c1], W[sr:sr + L, :])

                # Stagger the remaining one-time memsets behind early tiles.
                while next_memset < W_BUFS and next_memset <= t + 2:
                    memset_eng[next_memset].memset(ws[next_memset][:], 0.0)
                    next_memset += 1
    nc.compile()
    return nc


def kernel(input_state, fock_idx, fock_dim):
    input_state = np.asarray(input_state)
    idx = np.asarray(fock_idx).astype(np.int64)
    D = int(fock_dim)
    B, n, _ = input_state.shape

    nc = _build(idx, D, n)

    out = np.empty((B, D, D), dtype=input_state.dtype)
    for start in range(0, B, N_CORES):
        stop = min(start + N_CORES, B)
        in_maps = [
            {"rho": np.ascontiguousarray(input_state[b], dtype=np.float32)}
            for b in range(start, stop)
        ]
        res = run_bass_kernel_spmd(nc, in_maps,
                                   core_ids=list(range(stop - start)))
        for k, b in enumerate(range(start, stop)):
            out[b] = res.results[k]["out"]
    return out
